# revision 16
# baseline (speedup 1.0000x reference)
"""3-layer GAT on trn2, 8 NeuronCores (SPMD).

Strategy:
- Nodes are permuted and dealt to 8 cores (snake order by in-degree);
  each core owns a contiguous SHARD of table rows and the dst-blocks
  (<=128 dst nodes each) made from them.
- Per layer: each core transforms its shard (feat|el|er = h @ [W|Wl|Wr],
  PE matmuls on DMA-transposed h tiles), AllGather of the 512B-row
  fp16 table, then an edge phase over 4 src-index ranges (dma_gather is
  int16-indexed, so the 100352-row table is addressed via 4 base slices).
- Edge phase: per 1024-edge gather call: dma_gather of [feat|el] rows by
  src; a K=1 ones-matmul broadcasts the call's dst_local row into PSUM;
  one is_equal builds all transposed one-hots, another builds edge-major
  one-hots; per 128-edge chunk a small PE matmul (OneHotT @ er_block)
  delivers er per edge; w = exp(leaky_relu(el+er)); V' = [w*feat | w];
  main PE matmul OneHot^T @ V' accumulates [dst, 132] in PSUM per block.
- Per (block, pass): PSUM is added into a per-block SBUF accumulator;
  after the 4 passes an epilogue divides by the denominator, applies
  residual/ELU (or head-mean on the last layer) and writes h / output.
"""
import numpy as np

import concourse.bacc as bacc
import concourse.bass as bass
import concourse.mybir as mybir
import concourse.tile as tile
from concourse.bass_utils import run_bass_kernel_spmd

P = 128
NCORES = 8
HEADS = 4
F = 32
D = 128            # feature dim (all layers)
TW = 256           # table row elems fp16 (512B)
RANGE = 32768      # rows per gather base slice (int16 idx limit)
CALL_CH = 8        # chunks per gather call (1024 idxs)
fp16 = mybir.dt.float16
fp32 = mybir.dt.float32
AF = mybir.ActivationFunctionType
OP = mybir.AluOpType


# ----------------------------------------------------------------- host side
def _preprocess(src, dst, n_nodes):
    """Build node permutation, per-core schedules and index tiles."""
    E = src.shape[0]
    shard = int(np.ceil(n_nodes / (NCORES * P))) * P          # 12544
    ntot = NCORES * shard
    deg = np.bincount(dst, minlength=n_nodes)
    order = np.argsort(-deg, kind="stable")                    # desc degree
    # snake deal to cores
    core_of_pos = np.tile(np.concatenate([np.arange(NCORES),
                                          np.arange(NCORES)[::-1]]),
                          (n_nodes + 2 * NCORES - 1) // (2 * NCORES))[:n_nodes]
    node_core = np.empty(n_nodes, np.int64)
    node_core[order] = core_of_pos
    # position within core (by deal order -> desc degree within core)
    perm = np.empty(n_nodes, np.int64)                         # node -> table row
    core_nodes = []
    for c in range(NCORES):
        nodes_c = order[node_core[order] == c]
        core_nodes.append(nodes_c)
        perm[nodes_c] = c * shard + np.arange(len(nodes_c))
    row_of_node = perm
    # per-core blocks: consecutive 128 rows of the shard
    gmax = shard // P                                          # 98 blocks hold real nodes
    # group edges: core/block of dst, range of src
    e_core = node_core[dst]
    e_blk = (row_of_node[dst] % shard) // P
    e_rng = row_of_node[src] // RANGE
    e_idx = (row_of_node[src] % RANGE).astype(np.int16)
    e_dloc = (row_of_node[dst] % P).astype(np.int16)

    # counts[c, b, r]
    counts = np.zeros((NCORES, gmax, 4), np.int64)
    np.add.at(counts, (e_core, e_blk, e_rng), 1)
    # chunk table l[b, r] = max over cores (blocks are degree-sorted so the
    # per-position max is tight); at least 1 so every block gets a pass-0 copy
    l = np.maximum(np.ceil(counts / P).astype(np.int64).max(axis=0), 1)  # [g, 4]
    G = gmax
    chunks_r = l.sum(axis=0)                                   # per pass
    chmax = int(chunks_r.max())
    ncalls_r = [int(np.ceil(c / CALL_CH)) for c in chunks_r]
    callmax = max(ncalls_r)

    # per-core streams
    # slot start of (b, r) within pass r: prefix over b of l[:, r]
    starts = np.zeros((G, 4), np.int64)
    starts[1:] = np.cumsum(l[:-1], axis=0)

    idx_tiles = np.zeros((NCORES, 4, callmax, P, CALL_CH * P // 16), np.int16)
    dcol = np.full((NCORES, 4, P, chmax), 240, np.float16)
    oht = np.zeros((NCORES, 4, callmax, P, CALL_CH * P), np.float16)

    eo = np.lexsort((e_rng, e_blk, e_core))                    # group edges
    es, eb, er_, ei, ed = (x[eo] for x in (e_core, e_blk, e_rng, e_idx, e_dloc))
    # offsets within group via cumcount
    grp = es * (gmax * 4) + eb * 4 + er_
    sort_count = np.bincount(grp, minlength=NCORES * gmax * 4)
    within = np.arange(len(eo)) - np.repeat(
        np.concatenate([[0], np.cumsum(sort_count)[:-1]]), sort_count)

    slot = starts[eb, er_] * P + within                        # edge slot in pass
    # fill idx / dloc arrays
    for c in range(NCORES):
        m = es == c
        s, r, iv, dv = slot[m], er_[m], ei[m], ed[m]
        for rr in range(4):
            mm = r == rr
            ss = s[mm]
            flat_i = np.zeros(chunks_r[rr] * P, np.int16)
            flat_d = np.full(chunks_r[rr] * P, 240, np.int16)
            flat_i[ss] = iv[mm]
            flat_d[ss] = dv[mm]
            dcol[c, rr, :, :chunks_r[rr]] = (
                flat_d.reshape(chunks_r[rr], P).T.astype(np.float16))
            for k in range(ncalls_r[rr]):
                seg_d = flat_d[k * 1024:(k + 1) * 1024]
                oht[c, rr, k, :, :len(seg_d)] = (
                    seg_d[None, :] == np.arange(P)[:, None]).astype(np.float16)
            # idx tiles per call: i-th idx of call k at [16a+(i%16), i//16]
            for k in range(ncalls_r[rr]):
                seg = flat_i[k * 1024:(k + 1) * 1024]
                n = len(seg)
                if n < 1024:
                    seg = np.concatenate([seg, np.zeros(1024 - n, np.int16)])
                idx_tiles[c, rr, k] = np.tile(seg.reshape(64, 16).T, (8, 1))

    sched = dict(shard=shard, ntot=ntot, G=G, l=l, chunks_r=chunks_r,
                 ncalls_r=ncalls_r, callmax=callmax, chmax=chmax,
                 starts=starts)
    data = dict(idx_tiles=idx_tiles, dcol=dcol, oht=oht)
    return perm, core_nodes, sched, data


# ------------------------------------------------------------- device program
def _build(sched):
    shard, G = sched["shard"], sched["G"]
    l, chunks_r, ncalls_r = sched["l"], sched["chunks_r"], sched["ncalls_r"]
    callmax, chmax = sched["callmax"], sched["chmax"]
    GR = G * P                                                 # rows incl dummy blocks
    NT = sched["ntot"]

    nc = bacc.Bacc("TRN2", target_bir_lowering=False, debug=False,
                   num_devices=NCORES)
    feats = nc.dram_tensor("feats", [GR, D], fp16, kind="ExternalInput")
    wcat = nc.dram_tensor("wcat", [3, D, 136], fp16, kind="ExternalInput")
    idx_t = nc.dram_tensor("idx_t", [4, callmax, P, 64], mybir.dt.int16,
                           kind="ExternalInput")
    dcol_t = nc.dram_tensor("dcol_t", [4, P, chmax], fp16, kind="ExternalInput")
    oht_t = nc.dram_tensor("oht_t", [4, callmax, P, CALL_CH * P], fp16,
                           kind="ExternalInput")
    out_sh = nc.dram_tensor("out_sh", [GR, F], fp32, kind="ExternalOutput")

    # prefix sums for chunk -> block mapping per pass
    starts = sched["starts"]

    with tile.TileContext(nc) as tc:
        with (
            tc.tile_pool(name="const", bufs=1) as cp,
            tc.tile_pool(name="dram", bufs=1, space="DRAM") as dramp,
            tc.tile_pool(name="io", bufs=6) as iop,
            tc.tile_pool(name="gath", bufs=4) as gp,
            tc.tile_pool(name="oh", bufs=4) as ohp,
            tc.tile_pool(name="sm", bufs=4) as smp,
            tc.tile_pool(name="acc", bufs=G) as accp,
            tc.tile_pool(name="slab", bufs=2) as slabp,
            tc.tile_pool(name="ps", bufs=3, space="PSUM") as psp,
            tc.tile_pool(name="pse", bufs=3, space="PSUM") as psep,
        ):
            # DRAM scratch
            h1 = dramp.tile([GR, D], fp16)
            h2 = dramp.tile([GR, D], fp16)
            bounce = dramp.tile([GR, TW], fp16)
            table = dramp.tile([4 * RANGE, TW], fp16)

            # constants
            iota = cp.tile([P, P], fp16)
            nc.gpsimd.iota(iota[:], pattern=[[1, P]], base=0,
                           channel_multiplier=0,
                           allow_small_or_imprecise_dtypes=True)
            iota8 = cp.tile([P, CALL_CH, P], fp16)
            nc.gpsimd.iota(iota8[:], pattern=[[0, CALL_CH], [1, P]], base=0,
                           channel_multiplier=0,
                           allow_small_or_imprecise_dtypes=True)
            chiota = cp.tile([P, 1], fp32)
            nc.gpsimd.iota(chiota[:], pattern=[[0, 1]], base=0,
                           channel_multiplier=1,
                           allow_small_or_imprecise_dtypes=True)
            ones_row = cp.tile([1, P], fp16)
            nc.vector.memset(ones_row[:], 1.0)
            zeros = cp.tile([P, TW], fp16)
            nc.vector.memset(zeros[:], 0.0)
            # zero the dummy-block rows of bounce (er reads; avoid NaN)
            for b in range(shard // P, G):
                nc.sync.dma_start(bounce[b * P:(b + 1) * P, :], zeros[:])
            # zero the row at each range base: pad gather idxs point there
            for r in range(1, 4):
                nc.sync.dma_start(table[r * RANGE:r * RANGE + 1, :],
                                  zeros[0:1, :])

            wcs = []
            for L in range(3):
                wc = cp.tile([D, 136], fp16, tag="wc")
                nc.sync.dma_start(wc[:], wcat[L])
                wcs.append(wc)

            h_of = {0: feats[:], 1: h1[:], 2: h2[:]}

            for L in range(3):
                h_cur = h_of[L]
                # ---- transform: bounce[rows, 0:136] = hT.T @ wcat
                for t in range(shard // P):
                    hT = smp.tile([P, P], fp16, tag="hT")
                    nc.sync.dma_start(hT[:], h_cur[t * P:(t + 1) * P, :],
                                      transpose=True)
                    ptf = psp.tile([P, 136], fp32, space="PSUM", tag="pm")
                    nc.tensor.matmul(ptf[:], hT[:], wcs[L][:],
                                     start=True, stop=True)
                    stf = smp.tile([P, 136], fp16, tag="stf")
                    nc.vector.tensor_copy(stf[:], ptf[:])
                    nc.sync.dma_start(bounce[t * P:(t + 1) * P, 0:136], stf[:])

                # ---- allgather the padded-row table
                nc.gpsimd.collective_compute(
                    "AllGather", OP.bypass,
                    replica_groups=[list(range(NCORES))],
                    ins=[bounce[0:shard, :].opt()],
                    outs=[table[0:NT, :].opt()])

                # er per block, resident: er_all[p, b, h] = bounce[b*128+p, 132+h]
                er_all = smp.tile([P, G, 4], fp16, tag="er_all")
                nc.sync.dma_start(
                    er_all[:],
                    bounce[0:GR, 132:136].rearrange("(b p) h -> p b h", p=P))

                # per-block accumulators
                accs = [accp.tile([P, 132], fp32, tag="acc", name=f"acc{L}_{b}")
                        for b in range(G)]
                first_pass = [True] * G

                for r in range(4):
                    dcol_s = slabp.tile([P, chmax], fp16, tag="dcol")
                    nc.sync.dma_start(dcol_s[:], dcol_t[r])

                    nch = int(chunks_r[r])
                    # block segment boundaries in this pass
                    blk_of = np.repeat(np.arange(G), l[:, r])
                    pm_tiles = {}
                    for k in range(ncalls_r[r]):
                        c0 = k * CALL_CH
                        ncc = min(CALL_CH, nch - c0)
                        ni = ncc * P
                        it = iop.tile([P, 64], mybir.dt.int16, tag="idx")
                        nc.sync.dma_start(it[:], idx_t[r, k])
                        Gt = gp.tile([P, CALL_CH, TW], fp16, tag="G")
                        nc.gpsimd.dma_gather(
                            Gt[:, 0:ncc, :], table[r * RANGE:(r + 1) * RANGE, :],
                            it[:, 0:ni // 16], num_idxs=ni, num_idxs_reg=ni,
                            elem_size=TW)
                        OT = ohp.tile([P, CALL_CH, P], fp16, tag="OT")
                        nc.sync.dma_start(
                            OT[:, 0:ncc, :],
                            oht_t[r, k, :, 0:ni].rearrange(
                                "p (c e) -> p c e", e=P))
                        OE = ohp.tile([P, CALL_CH, P], fp16, tag="OE")
                        nc.vector.tensor_tensor(
                            out=OE[:, 0:ncc, :],
                            in0=iota8[:, 0:ncc, :],
                            in1=dcol_s[:, c0:c0 + ncc].unsqueeze(2)
                                .to_broadcast([P, ncc, P]),
                            op=OP.is_equal)
                        # er matmuls per chunk
                        erp = psep.tile([P, CALL_CH * 4], fp32, space="PSUM",
                                        tag="er")
                        for c in range(ncc):
                            b = int(blk_of[c0 + c])
                            nc.tensor.matmul(erp[:, c * 4:(c + 1) * 4],
                                             OT[:, c, :], er_all[:, b, :],
                                             start=True, stop=True)
                        er16 = smp.tile([P, CALL_CH * 4], fp16, tag="er16")
                        nc.scalar.activation(er16[:, 0:ncc * 4],
                                             erp[:, 0:ncc * 4], AF.Copy)
                        e32 = smp.tile([P, CALL_CH * 4], fp32, tag="e32")
                        nc.vector.tensor_tensor(
                            out=e32[:, 0:ncc * 4]
                                .rearrange("p (c h) -> p c h", h=4),
                            in0=Gt[:, 0:ncc, 128:132],
                            in1=er16[:, 0:ncc * 4]
                                .rearrange("p (c h) -> p c h", h=4),
                            op=OP.add)
                        lr = smp.tile([P, CALL_CH * 4], fp32, tag="lr")
                        nc.vector.tensor_scalar_mul(lr[:, 0:ncc * 4],
                                                    e32[:, 0:ncc * 4], 0.2)
                        nc.vector.tensor_tensor(out=lr[:, 0:ncc * 4],
                                                in0=lr[:, 0:ncc * 4],
                                                in1=e32[:, 0:ncc * 4], op=OP.max)
                        w16 = smp.tile([P, CALL_CH * 4], fp16, tag="w16")
                        nc.scalar.activation(w16[:, 0:ncc * 4], lr[:, 0:ncc * 4],
                                             AF.Exp)
                        V = gp.tile([P, CALL_CH, 132], fp16, tag="V")
                        nc.vector.tensor_tensor(
                            out=V[:, 0:ncc, 0:128]
                                .rearrange("p c (h f) -> p c h f", f=F),
                            in0=Gt[:, 0:ncc, 0:128]
                                .rearrange("p c (h f) -> p c h f", f=F),
                            in1=w16[:, 0:ncc * 4]
                                .rearrange("p (c h) -> p c h", h=4)
                                .unsqueeze(3).to_broadcast([P, ncc, 4, F]),
                            op=OP.mult)
                        nc.vector.tensor_copy(
                            V[:, 0:ncc, 128:132],
                            w16[:, 0:ncc * 4].rearrange("p (c h) -> p c h", h=4))
                        # main matmuls, accumulate per block
                        for c in range(ncc):
                            gc = c0 + c
                            b = int(blk_of[gc])
                            sb_, lb = int(starts[b, r]), int(l[b, r])
                            if b not in pm_tiles:
                                pm_tiles[b] = psp.tile(
                                    [P, 136], fp32, space="PSUM", tag="pm",
                                    name=f"pm{L}_{r}_{b}")
                            nc.tensor.matmul(pm_tiles[b][:, 0:132],
                                             OE[:, c, :], V[:, c, :],
                                             start=(gc == sb_),
                                             stop=(gc == sb_ + lb - 1))
                            if gc == sb_ + lb - 1:
                                if first_pass[b]:
                                    nc.vector.tensor_copy(
                                        accs[b][:], pm_tiles[b][:, 0:132])
                                    first_pass[b] = False
                                else:
                                    nc.vector.tensor_tensor(
                                        out=accs[b][:], in0=accs[b][:],
                                        in1=pm_tiles[b][:, 0:132], op=OP.add)
                                del pm_tiles[b]

                # ---- epilogue per block
                for b in range(G):
                    acc = accs[b]
                    rec = smp.tile([P, 4], fp32, tag="rec")
                    nc.vector.reciprocal(rec[:], acc[:, 128:132])
                    av = smp.tile([P, HEADS, F], fp32, tag="av")
                    nc.vector.tensor_tensor(
                        out=av[:],
                        in0=acc[:, 0:128].rearrange("p (h f) -> p h f", f=F),
                        in1=rec[:].unsqueeze(2).to_broadcast([P, HEADS, F]),
                        op=OP.mult)
                    if L < 2:
                        if L >= 1:  # residual (L1 adds h1)
                            hres = smp.tile([P, D], fp16, tag="hres")
                            nc.sync.dma_start(
                                hres[:], h_of[L][b * P:(b + 1) * P, :])
                            nc.vector.tensor_tensor(
                                out=av[:],
                                in0=av[:],
                                in1=hres[:].rearrange("p (h f) -> p h f", f=F),
                                op=OP.add)
                        # ELU: relu(x) + exp(min(x,0)) - 1
                        relu = smp.tile([P, D], fp32, tag="relu")
                        nc.vector.tensor_scalar_max(
                            relu[:], av[:].rearrange("p h f -> p (h f)"), 0.0)
                        mn = smp.tile([P, D], fp32, tag="mn")
                        nc.vector.tensor_scalar_min(
                            mn[:], av[:].rearrange("p h f -> p (h f)"), 0.0)
                        ex = smp.tile([P, D], fp32, tag="ex")
                        nc.scalar.activation(ex[:], mn[:], AF.Exp)
                        hnext = smp.tile([P, D], fp16, tag="hnext")
                        nc.vector.tensor_tensor(out=ex[:], in0=ex[:], in1=relu[:],
                                                op=OP.add)
                        nc.vector.tensor_scalar_add(hnext[:], ex[:], -1.0)
                        nc.sync.dma_start(
                            h_of[L + 1][b * P:(b + 1) * P, :], hnext[:])
                    else:
                        # residual + mean over heads
                        hres = smp.tile([P, D], fp16, tag="hres")
                        nc.sync.dma_start(hres[:],
                                          h_of[2][b * P:(b + 1) * P, :])
                        nc.vector.tensor_tensor(
                            out=av[:], in0=av[:],
                            in1=hres[:].rearrange("p (h f) -> p h f", f=F),
                            op=OP.add)
                        o32 = smp.tile([P, F], fp32, tag="o32")
                        nc.vector.tensor_tensor(out=o32[:], in0=av[:, 0, :],
                                                in1=av[:, 1, :], op=OP.add)
                        nc.vector.tensor_tensor(out=o32[:], in0=o32[:],
                                                in1=av[:, 2, :], op=OP.add)
                        nc.vector.tensor_tensor(out=o32[:], in0=o32[:],
                                                in1=av[:, 3, :], op=OP.add)
                        nc.vector.tensor_scalar_mul(o32[:], o32[:], 0.25)
                        nc.sync.dma_start(out_sh[b * P:(b + 1) * P, :], o32[:])
    nc.compile()
    return nc


_CACHE = {}
LAST_RESULTS = None


def kernel(**inputs):
    feats_f32 = np.asarray(inputs["features"], np.float32)
    src = np.asarray(inputs["src"]).astype(np.int64)
    dst = np.asarray(inputs["dst"]).astype(np.int64)
    n_nodes = feats_f32.shape[0]

    perm, core_nodes, sched, data = _preprocess(src, dst, n_nodes)
    shard, G = sched["shard"], sched["G"]

    # weights: Wcat[L] = [W | Wl | Wr] with Wl = sum_f W[:,h,f]*al[h,f]
    wcat = np.zeros((3, D, 136), np.float16)
    for L, (wn, an, bn) in enumerate([("W0", "al0", "ar0"),
                                      ("W1", "al1", "ar1"),
                                      ("W2", "al2", "ar2")]):
        W = np.asarray(inputs[wn], np.float32)
        al = np.asarray(inputs[an], np.float32)
        ar = np.asarray(inputs[bn], np.float32)
        Wh = W.reshape(D, HEADS, F)
        wcat[L, :, 0:128] = W.astype(np.float16)
        wcat[L, :, 128:132] = np.einsum("dhf,hf->dh", Wh, al).astype(np.float16)
        wcat[L, :, 132:136] = np.einsum("dhf,hf->dh", Wh, ar).astype(np.float16)

    key = (n_nodes, src.shape[0])
    if key not in _CACHE:
        _CACHE[key] = _build(sched)
    nc = _CACHE[key]

    feats16 = np.zeros((G * P, D), np.float16)
    in_maps = []
    for c in range(NCORES):
        f16 = np.zeros((G * P, D), np.float16)
        nodes_c = core_nodes[c]
        f16[:len(nodes_c)] = feats_f32[nodes_c].astype(np.float16)
        in_maps.append({
            "feats": f16,
            "wcat": wcat,
            "idx_t": data["idx_tiles"][c],
            "dcol_t": data["dcol"][c],
            "oht_t": data["oht"][c],
        })

    import os
    trace = bool(int(os.environ.get("TRN_KERNEL_TRACE", "0")))
    res = run_bass_kernel_spmd(nc, in_maps, core_ids=list(range(NCORES)),
                               trace=trace)
    global LAST_RESULTS
    LAST_RESULTS = res
    out = np.zeros((n_nodes, F), np.float32)
    for c in range(NCORES):
        nodes_c = core_nodes[c]
        out[nodes_c] = res.results[c]["out_sh"][:len(nodes_c)]
    return out
